# revision 13
# baseline (speedup 1.0000x reference)
"""EdgeMidpointEncoder Trainium2 kernel (8 NeuronCores, edge-sharded).

Strategy (per core, edges data-parallel across 8 cores):
  - Node feature table lives in HBM as unpadded bf16 rows (128B). dma_gather
    (transpose mode, elem = 256B = a pair of rows, idx = node>>1) delivers
    features directly in [feature, token] layout: partitions 0:63 = wanted
    row, 64:127 = the pair neighbor (junk, killed by zero weight rows).
    int16 idx limit (32767) is satisfied because idx = node>>1 < 25088; row
    parity is resolved by a per-group 128B base offset into the table, so
    edges are grouped host-side by (parity_i, parity_j) into 4 groups.
  - Layer 1: H[hid, tok] accumulated in PSUM over 3 K-chunks (feat_i,
    feat_j, |diff|+len), weights as zero-padded [128, 128] bf16 lhsT tiles.
  - ReLU+bias evac on ScalarE to bf16.
  - Layer 2 token-major: out2[tok, out] = H_block.T @ W2 -> outputs land
    token-major in PSUM, no transposes anywhere.
  - v0 = (amp + b2v) * [cos, sin](theta) via per-column broadcast multiply.
"""

import os
import sys

sys.path.insert(0, "/opt/trn_rl_repo")

import ml_dtypes
import numpy as np

import concourse.bass as bass
import concourse.mybir as mybir
import concourse.tile as tile
from concourse import bacc
from concourse.bass_utils import run_bass_kernel_spmd

P = 128
F = 64
HID = 128
SDIM = 64
VDIM = 32
N_CORES = 8
TBL_ROWS = 50176  # >= N_NODES+1, even
CHUNK = int(os.environ.get("CHUNK_OVERRIDE", "2048"))  # tokens per dma_gather
OUT_BF16 = os.environ.get("OUT_BF16", "0") == "1"
BF16 = ml_dtypes.bfloat16

_CACHE = {}


def _build(GSIZE, chunks, with_b2, repeat=1):
    dt = mybir.dt
    out_dt = dt.bfloat16 if OUT_BF16 else dt.float32
    nc = bacc.Bacc("TRN2", target_bir_lowering=False)
    table = nc.dram_tensor("table", [TBL_ROWS * F], dt.bfloat16, kind="ExternalInput")
    idx_i = nc.dram_tensor("idx_i", [4, P, GSIZE // 16], dt.int16, kind="ExternalInput")
    idx_j = nc.dram_tensor("idx_j", [4, P, GSIZE // 16], dt.int16, kind="ExternalInput")
    lens = nc.dram_tensor("lens", [4, GSIZE], dt.bfloat16, kind="ExternalInput")
    trig = nc.dram_tensor("trig", [4, 2, P, GSIZE // P], dt.float32, kind="ExternalInput")
    w1 = nc.dram_tensor("w1", [6, P, P], dt.bfloat16, kind="ExternalInput")
    w2s = nc.dram_tensor("w2s", [P, SDIM], dt.bfloat16, kind="ExternalInput")
    w2v = nc.dram_tensor("w2v", [P, VDIM], dt.bfloat16, kind="ExternalInput")
    b1 = nc.dram_tensor("b1", [2, P], dt.float32, kind="ExternalInput")
    b2s4 = nc.dram_tensor("b2s4", [P, 4 * SDIM], dt.float32, kind="ExternalInput")
    b2v4 = nc.dram_tensor("b2v4", [P, 4 * VDIM], dt.float32, kind="ExternalInput")
    h0o = nc.dram_tensor("h0o", [4, GSIZE, SDIM], out_dt, kind="ExternalOutput")
    v0o = nc.dram_tensor("v0o", [4, GSIZE, 2 * VDIM], out_dt, kind="ExternalOutput")

    n_pair_e = TBL_ROWS // 2
    n_pair_o = (TBL_ROWS * F - F) // (2 * F)
    in_even = table[: n_pair_e * 2 * F].rearrange("(n e) -> n e", e=2 * F)
    in_odd = table[F : F + n_pair_o * 2 * F].rearrange("(n e) -> n e", e=2 * F)

    AF = mybir.ActivationFunctionType
    OP = mybir.AluOpType

    with tile.TileContext(nc) as tc:
        with (
            tc.tile_pool(name="const", bufs=1) as cp,
            tc.tile_pool(name="gath", bufs=2) as gp,
            tc.tile_pool(name="work", bufs=3) as wp,
            tc.tile_pool(name="psumH", bufs=2, space="PSUM") as ph,
            tc.tile_pool(name="psumO", bufs=2, space="PSUM") as po_pool,
        ):
            w1_t = []
            for k in range(6):
                t = cp.tile([P, P], dt.bfloat16, name=f"w1_{k}")
                nc.sync.dma_start(t[:], w1[k])
                w1_t.append(t)
            w2s_t = cp.tile([P, SDIM], dt.bfloat16, name="w2s")
            nc.sync.dma_start(w2s_t[:], w2s[:])
            w2v_t = cp.tile([P, VDIM], dt.bfloat16, name="w2v")
            nc.sync.dma_start(w2v_t[:], w2v[:])
            b1_t = []
            for h in range(2):
                t = cp.tile([P, 1], dt.float32, name=f"b1_{h}")
                nc.sync.dma_start(t[:], b1[h, :, None])
                b1_t.append(t)
            if with_b2:
                b2s4_t = cp.tile([P, 4 * SDIM], dt.float32, name="b2s4")
                nc.sync.dma_start(b2s4_t[:], b2s4[:])
                b2v4_t = cp.tile([P, 4 * VDIM], dt.float32, name="b2v4")
                nc.sync.dma_start(b2v4_t[:], b2v4[:])

            def body():
                for g in range(4):
                    pi, pj = g >> 1, g & 1
                    in_i = in_odd if pi else in_even
                    in_j = in_odd if pj else in_even
                    col0 = 0
                    for csz in chunks:
                        ii = gp.tile([P, csz // 16], dt.int16, tag="ii", name="ii")
                        ij = gp.tile([P, csz // 16], dt.int16, tag="ij", name="ij")
                        nc.sync.dma_start(ii[:], idx_i[g, :, col0 // 16 : (col0 + csz) // 16])
                        nc.sync.dma_start(ij[:], idx_j[g, :, col0 // 16 : (col0 + csz) // 16])
                        gi = gp.tile([P, csz], dt.bfloat16, tag="gi", name="gi")
                        gj = gp.tile([P, csz], dt.bfloat16, tag="gj", name="gj")
                        nc.gpsimd.dma_gather(
                            gi[:, None, :], in_i, ii[:], csz, csz, 2 * F,
                            transpose=True, single_packet=False,
                        )
                        nc.gpsimd.dma_gather(
                            gj[:, None, :], in_j, ij[:], csz, csz, 2 * F,
                            transpose=True, single_packet=False,
                        )
                        cost = gp.tile([P, csz // P], dt.float32, tag="cost", name="cost")
                        sint = gp.tile([P, csz // P], dt.float32, tag="sint", name="sint")
                        nc.sync.dma_start(cost[:], trig[g, 0, :, col0 // P : (col0 + csz) // P])
                        nc.sync.dma_start(sint[:], trig[g, 1, :, col0 // P : (col0 + csz) // P])

                        for s in range(csz // 512):
                            sl = slice(s * 512, (s + 1) * 512)
                            dtile = wp.tile([P, 512], dt.bfloat16, tag="diff", name="diff")
                            nc.vector.tensor_tensor(dtile[:], gi[:, sl], gj[:, sl], op=OP.subtract)
                            du = dtile[:].bitcast(dt.uint16)
                            nc.vector.tensor_scalar(du, du, 0x7FFF, None, op0=OP.bitwise_and)
                            a0 = col0 + s * 512
                            nc.sync.dma_start(dtile[F : F + 1, :], lens[g, a0 : a0 + 512][None, :])

                            hp = [
                                ph.tile([P, 512], dt.float32, tag=f"H{h}", name=f"H{h}")
                                for h in range(2)
                            ]
                            for h in range(2):
                                nc.tensor.matmul(hp[h][:], w1_t[0 + h][:], gi[:, sl], start=True, stop=False)
                                nc.tensor.matmul(hp[h][:], w1_t[2 + h][:], gj[:, sl], start=False, stop=False)
                                nc.tensor.matmul(hp[h][:], w1_t[4 + h][:], dtile[:], start=False, stop=True)
                            hs = []
                            for h in range(2):
                                t = wp.tile([P, 512], dt.bfloat16, tag=f"hs{h}", name=f"hs{h}")
                                nc.scalar.activation(t[:], hp[h][:], AF.Relu, bias=b1_t[h][:, 0:1])
                                hs.append(t)

                            pso_t = po_pool.tile([P, 4 * SDIM], dt.float32, tag="pso", name="pso")
                            pa_t = po_pool.tile([P, 4 * VDIM], dt.float32, tag="pa", name="pa")
                            pso = pso_t[:]
                            pa = pa_t[:]
                            for b in range(4):
                                nc.tensor.matmul(
                                    pso[:, b * SDIM : (b + 1) * SDIM],
                                    hs[0][:, b * P : (b + 1) * P],
                                    w2s_t[:],
                                    start=True,
                                    stop=True,
                                )
                                nc.tensor.matmul(
                                    pa[:, b * VDIM : (b + 1) * VDIM],
                                    hs[1][:, b * P : (b + 1) * P],
                                    w2v_t[:],
                                    start=True,
                                    stop=True,
                                )

                            h0t = wp.tile([P, 4 * SDIM], out_dt, tag="h0t", name="h0t")
                            if with_b2:
                                nc.vector.tensor_tensor(h0t[:], pso, b2s4_t[:], op=OP.add)
                                pa_b = wp.tile([P, 4 * VDIM], dt.float32, tag="pa_b", name="pa_b")
                                nc.vector.tensor_tensor(pa_b[:], pa, b2v4_t[:], op=OP.add)
                                pa = pa_b[:]
                            else:
                                nc.scalar.copy(h0t[:], pso)
                            v0t = wp.tile([P, 4 * 2 * VDIM], out_dt, tag="v0t", name="v0t")
                            pa3 = pa.rearrange("p (b k) -> p b k", k=VDIM)
                            v03 = v0t[:].rearrange("p (b k c) -> p b k c", b=4, c=2)
                            cb = cost[:, s * 4 : (s + 1) * 4, None].to_broadcast([P, 4, VDIM])
                            sb = sint[:, s * 4 : (s + 1) * 4, None].to_broadcast([P, 4, VDIM])
                            nc.vector.tensor_tensor(v03[:, :, :, 0], pa3, cb, op=OP.mult)
                            nc.vector.tensor_tensor(v03[:, :, :, 1], pa3, sb, op=OP.mult)

                            out_h = h0o[g, a0 : a0 + 512, :].rearrange("(p b) f -> p (b f)", b=4)
                            out_v = v0o[g, a0 : a0 + 512, :].rearrange("(p b) f -> p (b f)", b=4)
                            nc.sync.dma_start(out_h, h0t[:])
                            nc.sync.dma_start(out_v, v0t[:])
                        col0 += csz

            if repeat > 1:
                with tc.For_i(0, repeat, 1):
                    body()
            else:
                body()

    nc.finalize()
    return nc


def _chunk_list(GSIZE):
    chunks = []
    rem = GSIZE
    while rem >= CHUNK:
        chunks.append(CHUNK)
        rem -= CHUNK
    if rem:
        chunks.append(rem)
    return chunks


def _wrap_idx(vals):
    # gather idx layout: partition p, col c holds vals[c*16 + p%16]
    a16 = vals.reshape(-1, 16).T  # [16, n/16]
    return np.tile(a16, (8, 1))  # [128, n/16]


def _prepare(
    endpoints,
    edge_lengths,
    midpoint_theta,
    endpoint_features,
    W1s, b1s, W2s, b2s, W1v, b1v, W2v, b2v,
    repeat=1,
):
    endpoints = np.asarray(endpoints)
    edge_lengths = np.asarray(edge_lengths, dtype=np.float32).reshape(-1)
    midpoint_theta = np.asarray(midpoint_theta, dtype=np.float32)
    endpoint_features = np.asarray(endpoint_features, dtype=np.float32)

    E = endpoints.shape[0]
    E_C = E // N_CORES
    ep = endpoints.astype(np.int64)

    # --- group edges by endpoint parity per core ---
    gids = ((ep[:, 0] & 1) * 2 + (ep[:, 1] & 1)).astype(np.int64)
    eids = [
        [np.nonzero(gids[c * E_C : (c + 1) * E_C] == g)[0] for g in range(4)]
        for c in range(N_CORES)
    ]
    gmax = max(len(x) for per in eids for x in per)
    GSIZE = ((gmax + 511) // 512) * 512
    chunks = tuple(_chunk_list(GSIZE))
    with_b2 = bool(np.any(b2s) or np.any(b2v))

    key = (GSIZE, chunks, with_b2, repeat, OUT_BF16)
    if key not in _CACHE:
        _CACHE[key] = _build(GSIZE, chunks, with_b2, repeat=repeat)
    nc = _CACHE[key]

    # --- shared (replicated) weight inputs ---
    table = np.zeros((TBL_ROWS, F), dtype=BF16)
    table[: endpoint_features.shape[0]] = endpoint_features.astype(BF16)
    W1cat = np.concatenate([np.asarray(W1s), np.asarray(W1v)], axis=1).astype(np.float32)
    w1_in = np.zeros((6, P, P), dtype=BF16)
    for ci, (r0, r1) in enumerate([(0, 64), (64, 128), (128, 193)]):
        for h in range(2):
            w1_in[ci * 2 + h, : r1 - r0, :] = W1cat[r0:r1, h * P : (h + 1) * P].astype(BF16)
    b1_in = np.concatenate([np.asarray(b1s), np.asarray(b1v)]).reshape(2, P).astype(np.float32)
    b2s4_in = np.broadcast_to(np.tile(np.asarray(b2s), 4)[None, :], (P, 4 * SDIM)).astype(np.float32)
    b2v4_in = np.broadcast_to(np.tile(np.asarray(b2v), 4)[None, :], (P, 4 * VDIM)).astype(np.float32)

    shared = {
        "table": table.reshape(-1),
        "w1": w1_in,
        "w2s": np.asarray(W2s).astype(BF16),
        "w2v": np.asarray(W2v).astype(BF16),
        "b1": b1_in,
        "b2s4": b2s4_in,
        "b2v4": b2v4_in,
    }

    # --- per-core permuted inputs ---
    r = np.arange(GSIZE)
    pos = (r // 512) * 512 + (r % 4) * 128 + (r % 512) // 4  # output-row -> gather-pos

    in_maps = []
    for c in range(N_CORES):
        sh = slice(c * E_C, (c + 1) * E_C)
        ep_c = ep[sh]
        len_c = edge_lengths[sh]
        th_c = midpoint_theta[sh]
        idx_i = np.zeros((4, P, GSIZE // 16), dtype=np.int16)
        idx_j = np.zeros((4, P, GSIZE // 16), dtype=np.int16)
        lens_in = np.zeros((4, GSIZE), dtype=BF16)
        trig_in = np.zeros((4, 2, P, GSIZE // P), dtype=np.float32)
        for g in range(4):
            ids = eids[c][g]
            L = len(ids)
            ivals = np.zeros(GSIZE, dtype=np.int16)
            jvals = np.zeros(GSIZE, dtype=np.int16)
            lvals = np.zeros(GSIZE, dtype=np.float32)
            tvals = np.zeros(GSIZE, dtype=np.float32)
            ivals[:L] = (ep_c[ids, 0] >> 1).astype(np.int16)
            jvals[:L] = (ep_c[ids, 1] >> 1).astype(np.int16)
            lvals[:L] = len_c[ids]
            tvals[:L] = th_c[ids]
            ivals_pos = np.zeros(GSIZE, dtype=np.int16)
            jvals_pos = np.zeros(GSIZE, dtype=np.int16)
            lvals_pos = np.zeros(GSIZE, dtype=np.float32)
            tvals_pos = np.zeros(GSIZE, dtype=np.float32)
            ivals_pos[pos] = ivals
            jvals_pos[pos] = jvals
            lvals_pos[pos] = lvals
            tvals_pos[pos] = tvals
            col0 = 0
            for csz in chunks:
                idx_i[g, :, col0 // 16 : (col0 + csz) // 16] = _wrap_idx(ivals_pos[col0 : col0 + csz])
                idx_j[g, :, col0 // 16 : (col0 + csz) // 16] = _wrap_idx(jvals_pos[col0 : col0 + csz])
                col0 += csz
            lens_in[g] = lvals_pos.astype(BF16)
            trig_in[g, 0] = np.cos(tvals_pos).reshape(GSIZE // P, P).T
            trig_in[g, 1] = np.sin(tvals_pos).reshape(GSIZE // P, P).T
        m = dict(shared)
        m["idx_i"] = idx_i
        m["idx_j"] = idx_j
        m["lens"] = lens_in
        m["trig"] = trig_in
        in_maps.append(m)

    meta = {"E": E, "E_C": E_C, "eids": eids}
    return nc, in_maps, meta


def _assemble(results, meta):
    E, E_C, eids = meta["E"], meta["E_C"], meta["eids"]
    h0 = np.empty((E, SDIM), dtype=np.float32)
    v0 = np.empty((E, 2 * VDIM), dtype=np.float32)
    for c in range(N_CORES):
        h0c = np.asarray(results[c]["h0o"], dtype=np.float32)
        v0c = np.asarray(results[c]["v0o"], dtype=np.float32)
        base = c * E_C
        for g in range(4):
            ids = eids[c][g]
            L = len(ids)
            h0[base + ids] = h0c[g, :L]
            v0[base + ids] = v0c[g, :L]
    return h0, v0.reshape(E, VDIM, 2)


def kernel(**inputs):
    trace = inputs.pop("trace", False)
    nc, in_maps, meta = _prepare(**inputs)
    res = run_bass_kernel_spmd(nc, in_maps, core_ids=list(range(N_CORES)), trace=trace)
    kernel.last_result = res
    return _assemble(res.results, meta)


# revision 14
# speedup vs baseline: 1.5010x; 1.5010x over previous
"""EdgeMidpointEncoder Trainium2 kernel (8 NeuronCores, edge-sharded).

Strategy (per core, edges data-parallel across 8 cores):
  - Node feature table lives in HBM as unpadded bf16 rows (128B). dma_gather
    (transpose mode, elem = 256B = a pair of rows, idx = node>>1) delivers
    features directly in [feature, token] layout: partitions 0:63 = wanted
    row, 64:127 = the pair neighbor (junk, killed by zero weight rows).
    int16 idx limit (32767) is satisfied because idx = node>>1 < 25088; row
    parity is resolved by a per-group 128B base offset into the table, so
    edges are grouped host-side by (parity_i, parity_j) into 4 groups.
  - Layer 1: H[hid, tok] accumulated in PSUM over 3 K-chunks (feat_i,
    feat_j, |diff|+len), weights as zero-padded [128, 128] bf16 lhsT tiles.
  - ReLU+bias evac on ScalarE to bf16.
  - Layer 2 token-major: out2[tok, out] = H_block.T @ W2 -> outputs land
    token-major in PSUM, no transposes anywhere.
  - v0 = (amp + b2v) * [cos, sin](theta) via per-column broadcast multiply.
"""

import os
import sys

sys.path.insert(0, "/opt/trn_rl_repo")

import ml_dtypes
import numpy as np

import concourse.bass as bass
import concourse.mybir as mybir
import concourse.tile as tile
from concourse import bacc
from concourse.bass_utils import run_bass_kernel_spmd

P = 128
F = 64
HID = 128
SDIM = 64
VDIM = 32
N_CORES = 8
TBL_ROWS = 50176  # >= N_NODES+1, even
CHUNK = int(os.environ.get("CHUNK_OVERRIDE", "2048"))  # tokens per dma_gather
OUT_BF16 = os.environ.get("OUT_BF16", "0") == "1"
SKIP_GATHER = os.environ.get("SKIP_GATHER", "0") == "1"
SKIP_OUT = os.environ.get("SKIP_OUT", "0") == "1"
SKIP_COMPUTE = os.environ.get("SKIP_COMPUTE", "0") == "1"
BF16 = ml_dtypes.bfloat16

_CACHE = {}


def _build(GSIZE, chunks, with_b2, repeat=1):
    dt = mybir.dt
    out_dt = dt.bfloat16 if OUT_BF16 else dt.float32
    nc = bacc.Bacc("TRN2", target_bir_lowering=False)
    table = nc.dram_tensor("table", [TBL_ROWS * F], dt.bfloat16, kind="ExternalInput")
    idx_i = nc.dram_tensor("idx_i", [4, P, GSIZE // 16], dt.int16, kind="ExternalInput")
    idx_j = nc.dram_tensor("idx_j", [4, P, GSIZE // 16], dt.int16, kind="ExternalInput")
    lens = nc.dram_tensor("lens", [4, GSIZE], dt.bfloat16, kind="ExternalInput")
    trig = nc.dram_tensor("trig", [4, 2, P, GSIZE // P], dt.float32, kind="ExternalInput")
    w1 = nc.dram_tensor("w1", [6, P, P], dt.bfloat16, kind="ExternalInput")
    w2s = nc.dram_tensor("w2s", [P, SDIM], dt.bfloat16, kind="ExternalInput")
    w2v = nc.dram_tensor("w2v", [P, VDIM], dt.bfloat16, kind="ExternalInput")
    b1 = nc.dram_tensor("b1", [2, P], dt.float32, kind="ExternalInput")
    b2s4 = nc.dram_tensor("b2s4", [P, 4 * SDIM], dt.float32, kind="ExternalInput")
    b2v4 = nc.dram_tensor("b2v4", [P, 4 * VDIM], dt.float32, kind="ExternalInput")
    h0o = nc.dram_tensor("h0o", [4, GSIZE, SDIM], out_dt, kind="ExternalOutput")
    v0o = nc.dram_tensor("v0o", [4, GSIZE, 2 * VDIM], out_dt, kind="ExternalOutput")

    n_pair_e = TBL_ROWS // 2
    n_pair_o = (TBL_ROWS * F - F) // (2 * F)
    in_even = table[: n_pair_e * 2 * F].rearrange("(n e) -> n e", e=2 * F)
    in_odd = table[F : F + n_pair_o * 2 * F].rearrange("(n e) -> n e", e=2 * F)

    AF = mybir.ActivationFunctionType
    OP = mybir.AluOpType

    with tile.TileContext(nc) as tc:
        with (
            tc.tile_pool(name="const", bufs=1) as cp,
            tc.tile_pool(name="gath", bufs=2) as gp,
            tc.tile_pool(name="work", bufs=3) as wp,
            tc.tile_pool(name="psumH", bufs=2, space="PSUM") as ph,
            tc.tile_pool(name="psumO", bufs=2, space="PSUM") as po_pool,
        ):
            w1_t = []
            for k in range(6):
                t = cp.tile([P, P], dt.bfloat16, name=f"w1_{k}")
                nc.sync.dma_start(t[:], w1[k])
                w1_t.append(t)
            w2s_t = cp.tile([P, SDIM], dt.bfloat16, name="w2s")
            nc.sync.dma_start(w2s_t[:], w2s[:])
            w2v_t = cp.tile([P, VDIM], dt.bfloat16, name="w2v")
            nc.sync.dma_start(w2v_t[:], w2v[:])
            b1_t = []
            for h in range(2):
                t = cp.tile([P, 1], dt.float32, name=f"b1_{h}")
                nc.sync.dma_start(t[:], b1[h, :, None])
                b1_t.append(t)
            if with_b2:
                b2s4_t = cp.tile([P, 4 * SDIM], dt.float32, name="b2s4")
                nc.sync.dma_start(b2s4_t[:], b2s4[:])
                b2v4_t = cp.tile([P, 4 * VDIM], dt.float32, name="b2v4")
                nc.sync.dma_start(b2v4_t[:], b2v4[:])

            def body():
                for g in range(4):
                    pi, pj = g >> 1, g & 1
                    in_i = in_odd if pi else in_even
                    in_j = in_odd if pj else in_even
                    col0 = 0
                    for csz in chunks:
                        ii = gp.tile([P, csz // 16], dt.int16, tag="ii", name="ii")
                        ij = gp.tile([P, csz // 16], dt.int16, tag="ij", name="ij")
                        nc.sync.dma_start(ii[:], idx_i[g, :, col0 // 16 : (col0 + csz) // 16])
                        nc.sync.dma_start(ij[:], idx_j[g, :, col0 // 16 : (col0 + csz) // 16])
                        gi = gp.tile([P, csz], dt.bfloat16, tag="gi", name="gi")
                        gj = gp.tile([P, csz], dt.bfloat16, tag="gj", name="gj")
                        if not SKIP_GATHER:
                            nc.gpsimd.dma_gather(
                                gi[:, None, :], in_i, ii[:], csz, csz, 2 * F,
                                transpose=True, single_packet=False,
                            )
                            nc.gpsimd.dma_gather(
                                gj[:, None, :], in_j, ij[:], csz, csz, 2 * F,
                                transpose=True, single_packet=False,
                            )
                        cost = gp.tile([P, csz // P], dt.float32, tag="cost", name="cost")
                        sint = gp.tile([P, csz // P], dt.float32, tag="sint", name="sint")
                        nc.sync.dma_start(cost[:], trig[g, 0, :, col0 // P : (col0 + csz) // P])
                        nc.sync.dma_start(sint[:], trig[g, 1, :, col0 // P : (col0 + csz) // P])

                        for s in range(csz // 512) if not SKIP_COMPUTE else []:
                            sl = slice(s * 512, (s + 1) * 512)
                            dtile = wp.tile([P, 512], dt.bfloat16, tag="diff", name="diff")
                            nc.vector.tensor_tensor(dtile[:], gi[:, sl], gj[:, sl], op=OP.subtract)
                            du = dtile[:].bitcast(dt.uint16)
                            nc.vector.tensor_scalar(du, du, 0x7FFF, None, op0=OP.bitwise_and)
                            a0 = col0 + s * 512
                            nc.sync.dma_start(dtile[F : F + 1, :], lens[g, a0 : a0 + 512][None, :])

                            hp = [
                                ph.tile([P, 512], dt.float32, tag=f"H{h}", name=f"H{h}")
                                for h in range(2)
                            ]
                            for h in range(2):
                                nc.tensor.matmul(hp[h][:], w1_t[0 + h][:], gi[:, sl], start=True, stop=False)
                                nc.tensor.matmul(hp[h][:], w1_t[2 + h][:], gj[:, sl], start=False, stop=False)
                                nc.tensor.matmul(hp[h][:], w1_t[4 + h][:], dtile[:], start=False, stop=True)
                            hs = []
                            for h in range(2):
                                t = wp.tile([P, 512], dt.bfloat16, tag=f"hs{h}", name=f"hs{h}")
                                nc.scalar.activation(t[:], hp[h][:], AF.Relu, bias=b1_t[h][:, 0:1])
                                hs.append(t)

                            pso_t = po_pool.tile([P, 4 * SDIM], dt.float32, tag="pso", name="pso")
                            pa_t = po_pool.tile([P, 4 * VDIM], dt.float32, tag="pa", name="pa")
                            pso = pso_t[:]
                            pa = pa_t[:]
                            for b in range(4):
                                nc.tensor.matmul(
                                    pso[:, b * SDIM : (b + 1) * SDIM],
                                    hs[0][:, b * P : (b + 1) * P],
                                    w2s_t[:],
                                    start=True,
                                    stop=True,
                                )
                                nc.tensor.matmul(
                                    pa[:, b * VDIM : (b + 1) * VDIM],
                                    hs[1][:, b * P : (b + 1) * P],
                                    w2v_t[:],
                                    start=True,
                                    stop=True,
                                )

                            h0t = wp.tile([P, 4 * SDIM], out_dt, tag="h0t", name="h0t")
                            if with_b2:
                                nc.vector.tensor_tensor(h0t[:], pso, b2s4_t[:], op=OP.add)
                                pa_b = wp.tile([P, 4 * VDIM], dt.float32, tag="pa_b", name="pa_b")
                                nc.vector.tensor_tensor(pa_b[:], pa, b2v4_t[:], op=OP.add)
                                pa = pa_b[:]
                            else:
                                nc.scalar.copy(h0t[:], pso)
                            v0t = wp.tile([P, 4 * 2 * VDIM], out_dt, tag="v0t", name="v0t")
                            pa3 = pa.rearrange("p (b k) -> p b k", k=VDIM)
                            v03 = v0t[:].rearrange("p (b k c) -> p b k c", b=4, c=2)
                            cb = cost[:, s * 4 : (s + 1) * 4, None].to_broadcast([P, 4, VDIM])
                            sb = sint[:, s * 4 : (s + 1) * 4, None].to_broadcast([P, 4, VDIM])
                            nc.vector.tensor_tensor(v03[:, :, :, 0], pa3, cb, op=OP.mult)
                            nc.vector.tensor_tensor(v03[:, :, :, 1], pa3, sb, op=OP.mult)

                            out_h = h0o[g, a0 : a0 + 512, :].rearrange("(p b) f -> p (b f)", b=4)
                            out_v = v0o[g, a0 : a0 + 512, :].rearrange("(p b) f -> p (b f)", b=4)
                            if not SKIP_OUT:
                                nc.sync.dma_start(out_h, h0t[:])
                                nc.sync.dma_start(out_v, v0t[:])
                        col0 += csz

            if repeat > 1:
                with tc.For_i(0, repeat, 1):
                    body()
            else:
                body()

    nc.finalize()
    return nc


def _chunk_list(GSIZE):
    chunks = []
    rem = GSIZE
    while rem >= CHUNK:
        chunks.append(CHUNK)
        rem -= CHUNK
    if rem:
        chunks.append(rem)
    return chunks


def _wrap_idx(vals):
    # gather idx layout: partition p, col c holds vals[c*16 + p%16]
    a16 = vals.reshape(-1, 16).T  # [16, n/16]
    return np.tile(a16, (8, 1))  # [128, n/16]


def _prepare(
    endpoints,
    edge_lengths,
    midpoint_theta,
    endpoint_features,
    W1s, b1s, W2s, b2s, W1v, b1v, W2v, b2v,
    repeat=1,
):
    endpoints = np.asarray(endpoints)
    edge_lengths = np.asarray(edge_lengths, dtype=np.float32).reshape(-1)
    midpoint_theta = np.asarray(midpoint_theta, dtype=np.float32)
    endpoint_features = np.asarray(endpoint_features, dtype=np.float32)

    E = endpoints.shape[0]
    E_C = E // N_CORES
    ep = endpoints.astype(np.int64)

    # --- group edges by endpoint parity per core ---
    gids = ((ep[:, 0] & 1) * 2 + (ep[:, 1] & 1)).astype(np.int64)
    eids = [
        [np.nonzero(gids[c * E_C : (c + 1) * E_C] == g)[0] for g in range(4)]
        for c in range(N_CORES)
    ]
    gmax = max(len(x) for per in eids for x in per)
    GSIZE = ((gmax + 511) // 512) * 512
    chunks = tuple(_chunk_list(GSIZE))
    with_b2 = bool(np.any(b2s) or np.any(b2v))

    key = (GSIZE, chunks, with_b2, repeat, OUT_BF16, SKIP_GATHER, SKIP_OUT, SKIP_COMPUTE)
    if key not in _CACHE:
        _CACHE[key] = _build(GSIZE, chunks, with_b2, repeat=repeat)
    nc = _CACHE[key]

    # --- shared (replicated) weight inputs ---
    table = np.zeros((TBL_ROWS, F), dtype=BF16)
    table[: endpoint_features.shape[0]] = endpoint_features.astype(BF16)
    W1cat = np.concatenate([np.asarray(W1s), np.asarray(W1v)], axis=1).astype(np.float32)
    w1_in = np.zeros((6, P, P), dtype=BF16)
    for ci, (r0, r1) in enumerate([(0, 64), (64, 128), (128, 193)]):
        for h in range(2):
            w1_in[ci * 2 + h, : r1 - r0, :] = W1cat[r0:r1, h * P : (h + 1) * P].astype(BF16)
    b1_in = np.concatenate([np.asarray(b1s), np.asarray(b1v)]).reshape(2, P).astype(np.float32)
    b2s4_in = np.broadcast_to(np.tile(np.asarray(b2s), 4)[None, :], (P, 4 * SDIM)).astype(np.float32)
    b2v4_in = np.broadcast_to(np.tile(np.asarray(b2v), 4)[None, :], (P, 4 * VDIM)).astype(np.float32)

    shared = {
        "table": table.reshape(-1),
        "w1": w1_in,
        "w2s": np.asarray(W2s).astype(BF16),
        "w2v": np.asarray(W2v).astype(BF16),
        "b1": b1_in,
        "b2s4": b2s4_in,
        "b2v4": b2v4_in,
    }

    # --- per-core permuted inputs ---
    r = np.arange(GSIZE)
    pos = (r // 512) * 512 + (r % 4) * 128 + (r % 512) // 4  # output-row -> gather-pos

    in_maps = []
    for c in range(N_CORES):
        sh = slice(c * E_C, (c + 1) * E_C)
        ep_c = ep[sh]
        len_c = edge_lengths[sh]
        th_c = midpoint_theta[sh]
        idx_i = np.zeros((4, P, GSIZE // 16), dtype=np.int16)
        idx_j = np.zeros((4, P, GSIZE // 16), dtype=np.int16)
        lens_in = np.zeros((4, GSIZE), dtype=BF16)
        trig_in = np.zeros((4, 2, P, GSIZE // P), dtype=np.float32)
        for g in range(4):
            ids = eids[c][g]
            L = len(ids)
            ivals = np.zeros(GSIZE, dtype=np.int16)
            jvals = np.zeros(GSIZE, dtype=np.int16)
            lvals = np.zeros(GSIZE, dtype=np.float32)
            tvals = np.zeros(GSIZE, dtype=np.float32)
            ivals[:L] = (ep_c[ids, 0] >> 1).astype(np.int16)
            jvals[:L] = (ep_c[ids, 1] >> 1).astype(np.int16)
            lvals[:L] = len_c[ids]
            tvals[:L] = th_c[ids]
            ivals_pos = np.zeros(GSIZE, dtype=np.int16)
            jvals_pos = np.zeros(GSIZE, dtype=np.int16)
            lvals_pos = np.zeros(GSIZE, dtype=np.float32)
            tvals_pos = np.zeros(GSIZE, dtype=np.float32)
            ivals_pos[pos] = ivals
            jvals_pos[pos] = jvals
            lvals_pos[pos] = lvals
            tvals_pos[pos] = tvals
            col0 = 0
            for csz in chunks:
                idx_i[g, :, col0 // 16 : (col0 + csz) // 16] = _wrap_idx(ivals_pos[col0 : col0 + csz])
                idx_j[g, :, col0 // 16 : (col0 + csz) // 16] = _wrap_idx(jvals_pos[col0 : col0 + csz])
                col0 += csz
            lens_in[g] = lvals_pos.astype(BF16)
            trig_in[g, 0] = np.cos(tvals_pos).reshape(GSIZE // P, P).T
            trig_in[g, 1] = np.sin(tvals_pos).reshape(GSIZE // P, P).T
        m = dict(shared)
        m["idx_i"] = idx_i
        m["idx_j"] = idx_j
        m["lens"] = lens_in
        m["trig"] = trig_in
        in_maps.append(m)

    meta = {"E": E, "E_C": E_C, "eids": eids}
    return nc, in_maps, meta


def _assemble(results, meta):
    E, E_C, eids = meta["E"], meta["E_C"], meta["eids"]
    h0 = np.empty((E, SDIM), dtype=np.float32)
    v0 = np.empty((E, 2 * VDIM), dtype=np.float32)
    for c in range(N_CORES):
        h0c = np.asarray(results[c]["h0o"], dtype=np.float32)
        v0c = np.asarray(results[c]["v0o"], dtype=np.float32)
        base = c * E_C
        for g in range(4):
            ids = eids[c][g]
            L = len(ids)
            h0[base + ids] = h0c[g, :L]
            v0[base + ids] = v0c[g, :L]
    return h0, v0.reshape(E, VDIM, 2)


def kernel(**inputs):
    trace = inputs.pop("trace", False)
    nc, in_maps, meta = _prepare(**inputs)
    res = run_bass_kernel_spmd(nc, in_maps, core_ids=list(range(N_CORES)), trace=trace)
    kernel.last_result = res
    return _assemble(res.results, meta)


# revision 15
# speedup vs baseline: 3.4497x; 2.2982x over previous
"""EdgeMidpointEncoder Trainium2 kernel (8 NeuronCores, edge-sharded).

Strategy (per core, edges data-parallel across 8 cores):
  - Node feature table lives in HBM as unpadded bf16 rows (128B). dma_gather
    (transpose mode, elem = 256B = a pair of rows, idx = node>>1) delivers
    features directly in [feature, token] layout: partitions 0:63 = wanted
    row, 64:127 = the pair neighbor (junk, killed by zero weight rows).
    int16 idx limit (32767) is satisfied because idx = node>>1 < 25088; row
    parity is resolved by a per-group 128B base offset into the table, so
    edges are grouped host-side by (parity_i, parity_j) into 4 groups.
  - Layer 1: H[hid, tok] accumulated in PSUM over 3 K-chunks (feat_i,
    feat_j, |diff|+len), weights as zero-padded [128, 128] bf16 lhsT tiles.
  - ReLU+bias evac on ScalarE to bf16.
  - Layer 2 token-major: out2[tok, out] = H_block.T @ W2 -> outputs land
    token-major in PSUM, no transposes anywhere.
  - v0 = (amp + b2v) * [cos, sin](theta) via per-column broadcast multiply.
"""

import os
import sys

sys.path.insert(0, "/opt/trn_rl_repo")

import ml_dtypes
import numpy as np

import concourse.bass as bass
import concourse.mybir as mybir
import concourse.tile as tile
from concourse import bacc
from concourse.bass_utils import run_bass_kernel_spmd

P = 128
F = 64
HID = 128
SDIM = 64
VDIM = 32
N_CORES = 8
TBL_ROWS = 50176  # >= N_NODES+1, even
CHUNK = int(os.environ.get("CHUNK_OVERRIDE", "2048"))  # tokens per dma_gather
OUT_BF16 = os.environ.get("OUT_BF16", "0") == "1"
SKIP_GATHER = os.environ.get("SKIP_GATHER", "0") == "1"
SKIP_OUT = os.environ.get("SKIP_OUT", "0") == "1"
SKIP_COMPUTE = os.environ.get("SKIP_COMPUTE", "0") == "1"
STAGE = int(os.environ.get("STAGE", "4"))  # 1=L1mm 2=+relu 3=+L2mm 4=+outputs
GQ = int(os.environ.get("GQ", "1"))  # number of SWDGE queues for gathers
GNT = os.environ.get("GNT", "0") == "1"  # timing expt: non-transpose gathers
BF16 = ml_dtypes.bfloat16

_CACHE = {}


def _build(GSIZE, chunks, with_b2, repeat=1):
    dt = mybir.dt
    out_dt = dt.bfloat16 if OUT_BF16 else dt.float32
    nc = bacc.Bacc("TRN2", target_bir_lowering=False, num_swdge_queues=GQ)
    table = nc.dram_tensor("table", [TBL_ROWS * F], dt.bfloat16, kind="ExternalInput")
    idx_i = nc.dram_tensor("idx_i", [4, P, GSIZE // 16], dt.int16, kind="ExternalInput")
    idx_j = nc.dram_tensor("idx_j", [4, P, GSIZE // 16], dt.int16, kind="ExternalInput")
    lens = nc.dram_tensor("lens", [4, GSIZE], dt.bfloat16, kind="ExternalInput")
    trig = nc.dram_tensor("trig", [4, 2, P, GSIZE // P], dt.float32, kind="ExternalInput")
    w1 = nc.dram_tensor("w1", [6, P, P], dt.bfloat16, kind="ExternalInput")
    w2s = nc.dram_tensor("w2s", [P, SDIM], dt.bfloat16, kind="ExternalInput")
    w2v = nc.dram_tensor("w2v", [P, VDIM], dt.bfloat16, kind="ExternalInput")
    b1 = nc.dram_tensor("b1", [2, P], dt.float32, kind="ExternalInput")
    b2s4 = nc.dram_tensor("b2s4", [P, 4 * SDIM], dt.float32, kind="ExternalInput")
    b2v4 = nc.dram_tensor("b2v4", [P, 4 * VDIM], dt.float32, kind="ExternalInput")
    h0o = nc.dram_tensor("h0o", [4, GSIZE, SDIM], out_dt, kind="ExternalOutput")
    v0o = nc.dram_tensor("v0o", [4, GSIZE, 2 * VDIM], out_dt, kind="ExternalOutput")

    n_pair_e = TBL_ROWS // 2
    n_pair_o = (TBL_ROWS * F - F) // (2 * F)
    in_even = table[: n_pair_e * 2 * F].rearrange("(n e) -> n e", e=2 * F)
    in_odd = table[F : F + n_pair_o * 2 * F].rearrange("(n e) -> n e", e=2 * F)

    AF = mybir.ActivationFunctionType
    OP = mybir.AluOpType

    with tile.TileContext(nc) as tc:
        with (
            tc.tile_pool(name="const", bufs=1) as cp,
            tc.tile_pool(name="gath", bufs=2) as gp,
            tc.tile_pool(name="work", bufs=3) as wp,
            tc.tile_pool(name="psumH", bufs=2, space="PSUM") as ph,
            tc.tile_pool(name="psumO", bufs=2, space="PSUM") as po_pool,
        ):
            w1_t = []
            for k in range(6):
                t = cp.tile([P, P], dt.bfloat16, name=f"w1_{k}")
                nc.sync.dma_start(t[:], w1[k])
                w1_t.append(t)
            w2s_t = cp.tile([P, SDIM], dt.bfloat16, name="w2s")
            nc.sync.dma_start(w2s_t[:], w2s[:])
            w2v_t = cp.tile([P, VDIM], dt.bfloat16, name="w2v")
            nc.sync.dma_start(w2v_t[:], w2v[:])
            b1_t = []
            for h in range(2):
                t = cp.tile([P, 1], dt.float32, name=f"b1_{h}")
                nc.sync.dma_start(t[:], b1[h, :, None])
                b1_t.append(t)
            if with_b2:
                b2s4_t = cp.tile([P, 4 * SDIM], dt.float32, name="b2s4")
                nc.sync.dma_start(b2s4_t[:], b2s4[:])
                b2v4_t = cp.tile([P, 4 * VDIM], dt.float32, name="b2v4")
                nc.sync.dma_start(b2v4_t[:], b2v4[:])

            def body():
                body.chunk_no = 0
                for g in range(4):
                    pi, pj = g >> 1, g & 1
                    in_i = in_odd if pi else in_even
                    in_j = in_odd if pj else in_even
                    col0 = 0
                    for csz in chunks:
                        ii = gp.tile([P, csz // 16], dt.int16, tag="ii", name="ii")
                        ij = gp.tile([P, csz // 16], dt.int16, tag="ij", name="ij")
                        nc.sync.dma_start(ii[:], idx_i[g, :, col0 // 16 : (col0 + csz) // 16])
                        nc.sync.dma_start(ij[:], idx_j[g, :, col0 // 16 : (col0 + csz) // 16])
                        gi = gp.tile([P, csz], dt.bfloat16, tag="gi", name="gi")
                        gj = gp.tile([P, csz], dt.bfloat16, tag="gj", name="gj")
                        qn = body.chunk_no % GQ
                        body.chunk_no += 1
                        if GNT:
                            nc.gpsimd.dma_gather(
                                gi[:].rearrange("p (b e) -> p b e", e=2 * F), in_i, ii[:],
                                csz, csz, 2 * F, transpose=False, single_packet=False,
                                queue_num=qn,
                            )
                            nc.gpsimd.dma_gather(
                                gj[:].rearrange("p (b e) -> p b e", e=2 * F), in_j, ij[:],
                                csz, csz, 2 * F, transpose=False, single_packet=False,
                                queue_num=(qn + 1) % GQ if GQ > 1 else qn,
                            )
                        elif not SKIP_GATHER:
                            nc.gpsimd.dma_gather(
                                gi[:, None, :], in_i, ii[:], csz, csz, 2 * F,
                                transpose=True, single_packet=False, queue_num=qn,
                            )
                            nc.gpsimd.dma_gather(
                                gj[:, None, :], in_j, ij[:], csz, csz, 2 * F,
                                transpose=True, single_packet=False,
                                queue_num=(qn + 1) % GQ if GQ > 1 else qn,
                            )
                        cost = gp.tile([P, csz // P], dt.float32, tag="cost", name="cost")
                        sint = gp.tile([P, csz // P], dt.float32, tag="sint", name="sint")
                        nc.sync.dma_start(cost[:], trig[g, 0, :, col0 // P : (col0 + csz) // P])
                        nc.sync.dma_start(sint[:], trig[g, 1, :, col0 // P : (col0 + csz) // P])

                        for s in range(csz // 512) if not SKIP_COMPUTE else []:
                            sl = slice(s * 512, (s + 1) * 512)
                            dtile = wp.tile([P, 512], dt.bfloat16, tag="diff", name="diff")
                            nc.vector.tensor_tensor(dtile[:], gi[:, sl], gj[:, sl], op=OP.subtract)
                            du = dtile[:].bitcast(dt.uint16)
                            nc.vector.tensor_scalar(du, du, 0x7FFF, None, op0=OP.bitwise_and)
                            a0 = col0 + s * 512
                            nc.sync.dma_start(dtile[F : F + 1, :], lens[g, a0 : a0 + 512][None, :])

                            if STAGE < 1:
                                continue
                            hp = [
                                ph.tile([P, 512], dt.float32, tag=f"H{h}", name=f"H{h}")
                                for h in range(2)
                            ]
                            for h in range(2):
                                nc.tensor.matmul(hp[h][:], w1_t[0 + h][:], gi[:, sl], start=True, stop=False)
                                nc.tensor.matmul(hp[h][:], w1_t[2 + h][:], gj[:, sl], start=False, stop=False)
                                nc.tensor.matmul(hp[h][:], w1_t[4 + h][:], dtile[:], start=False, stop=True)
                            if STAGE < 2:
                                continue
                            hs = []
                            for h in range(2):
                                t = wp.tile([P, 512], dt.bfloat16, tag=f"hs{h}", name=f"hs{h}")
                                nc.scalar.activation(t[:], hp[h][:], AF.Relu, bias=b1_t[h][:, 0:1])
                                hs.append(t)

                            if STAGE < 3:
                                continue
                            pso_t = po_pool.tile([P, 4 * SDIM], dt.float32, tag="pso", name="pso")
                            pa_t = po_pool.tile([P, 4 * VDIM], dt.float32, tag="pa", name="pa")
                            pso = pso_t[:]
                            pa = pa_t[:]
                            for b in range(4):
                                nc.tensor.matmul(
                                    pso[:, b * SDIM : (b + 1) * SDIM],
                                    hs[0][:, b * P : (b + 1) * P],
                                    w2s_t[:],
                                    start=True,
                                    stop=True,
                                )
                                nc.tensor.matmul(
                                    pa[:, b * VDIM : (b + 1) * VDIM],
                                    hs[1][:, b * P : (b + 1) * P],
                                    w2v_t[:],
                                    start=True,
                                    stop=True,
                                )

                            if STAGE < 4:
                                continue
                            h0t = wp.tile([P, 4 * SDIM], out_dt, tag="h0t", name="h0t")
                            if with_b2:
                                nc.vector.tensor_tensor(h0t[:], pso, b2s4_t[:], op=OP.add)
                                pa_b = wp.tile([P, 4 * VDIM], dt.float32, tag="pa_b", name="pa_b")
                                nc.vector.tensor_tensor(pa_b[:], pa, b2v4_t[:], op=OP.add)
                                pa = pa_b[:]
                            else:
                                nc.scalar.copy(h0t[:], pso)
                            v0t = wp.tile([P, 4 * 2 * VDIM], out_dt, tag="v0t", name="v0t")
                            pa3 = pa.rearrange("p (b k) -> p b k", k=VDIM)
                            v03 = v0t[:].rearrange("p (b k c) -> p b k c", b=4, c=2)
                            cb = cost[:, s * 4 : (s + 1) * 4, None].to_broadcast([P, 4, VDIM])
                            sb = sint[:, s * 4 : (s + 1) * 4, None].to_broadcast([P, 4, VDIM])
                            nc.vector.tensor_tensor(v03[:, :, :, 0], pa3, cb, op=OP.mult)
                            nc.vector.tensor_tensor(v03[:, :, :, 1], pa3, sb, op=OP.mult)

                            out_h = h0o[g, a0 : a0 + 512, :].rearrange("(p b) f -> p (b f)", b=4)
                            out_v = v0o[g, a0 : a0 + 512, :].rearrange("(p b) f -> p (b f)", b=4)
                            if not SKIP_OUT:
                                nc.sync.dma_start(out_h, h0t[:])
                                nc.sync.dma_start(out_v, v0t[:])
                        col0 += csz

            if repeat > 1:
                with tc.For_i(0, repeat, 1):
                    body()
            else:
                body()

    nc.finalize()
    return nc


def _chunk_list(GSIZE):
    chunks = []
    rem = GSIZE
    while rem >= CHUNK:
        chunks.append(CHUNK)
        rem -= CHUNK
    if rem:
        chunks.append(rem)
    return chunks


def _wrap_idx(vals):
    # gather idx layout: partition p, col c holds vals[c*16 + p%16]
    a16 = vals.reshape(-1, 16).T  # [16, n/16]
    return np.tile(a16, (8, 1))  # [128, n/16]


def _prepare(
    endpoints,
    edge_lengths,
    midpoint_theta,
    endpoint_features,
    W1s, b1s, W2s, b2s, W1v, b1v, W2v, b2v,
    repeat=1,
):
    endpoints = np.asarray(endpoints)
    edge_lengths = np.asarray(edge_lengths, dtype=np.float32).reshape(-1)
    midpoint_theta = np.asarray(midpoint_theta, dtype=np.float32)
    endpoint_features = np.asarray(endpoint_features, dtype=np.float32)

    E = endpoints.shape[0]
    E_C = E // N_CORES
    ep = endpoints.astype(np.int64)

    # --- group edges by endpoint parity per core ---
    gids = ((ep[:, 0] & 1) * 2 + (ep[:, 1] & 1)).astype(np.int64)
    eids = [
        [np.nonzero(gids[c * E_C : (c + 1) * E_C] == g)[0] for g in range(4)]
        for c in range(N_CORES)
    ]
    gmax = max(len(x) for per in eids for x in per)
    GSIZE = ((gmax + 511) // 512) * 512
    chunks = tuple(_chunk_list(GSIZE))
    with_b2 = bool(np.any(b2s) or np.any(b2v))

    key = (GSIZE, chunks, with_b2, repeat, OUT_BF16, SKIP_GATHER, SKIP_OUT, SKIP_COMPUTE, STAGE, GQ, GNT)
    if key not in _CACHE:
        _CACHE[key] = _build(GSIZE, chunks, with_b2, repeat=repeat)
    nc = _CACHE[key]

    # --- shared (replicated) weight inputs ---
    table = np.zeros((TBL_ROWS, F), dtype=BF16)
    table[: endpoint_features.shape[0]] = endpoint_features.astype(BF16)
    W1cat = np.concatenate([np.asarray(W1s), np.asarray(W1v)], axis=1).astype(np.float32)
    w1_in = np.zeros((6, P, P), dtype=BF16)
    for ci, (r0, r1) in enumerate([(0, 64), (64, 128), (128, 193)]):
        for h in range(2):
            w1_in[ci * 2 + h, : r1 - r0, :] = W1cat[r0:r1, h * P : (h + 1) * P].astype(BF16)
    b1_in = np.concatenate([np.asarray(b1s), np.asarray(b1v)]).reshape(2, P).astype(np.float32)
    b2s4_in = np.broadcast_to(np.tile(np.asarray(b2s), 4)[None, :], (P, 4 * SDIM)).astype(np.float32)
    b2v4_in = np.broadcast_to(np.tile(np.asarray(b2v), 4)[None, :], (P, 4 * VDIM)).astype(np.float32)

    shared = {
        "table": table.reshape(-1),
        "w1": w1_in,
        "w2s": np.asarray(W2s).astype(BF16),
        "w2v": np.asarray(W2v).astype(BF16),
        "b1": b1_in,
        "b2s4": b2s4_in,
        "b2v4": b2v4_in,
    }

    # --- per-core permuted inputs ---
    r = np.arange(GSIZE)
    pos = (r // 512) * 512 + (r % 4) * 128 + (r % 512) // 4  # output-row -> gather-pos

    in_maps = []
    for c in range(N_CORES):
        sh = slice(c * E_C, (c + 1) * E_C)
        ep_c = ep[sh]
        len_c = edge_lengths[sh]
        th_c = midpoint_theta[sh]
        idx_i = np.zeros((4, P, GSIZE // 16), dtype=np.int16)
        idx_j = np.zeros((4, P, GSIZE // 16), dtype=np.int16)
        lens_in = np.zeros((4, GSIZE), dtype=BF16)
        trig_in = np.zeros((4, 2, P, GSIZE // P), dtype=np.float32)
        for g in range(4):
            ids = eids[c][g]
            L = len(ids)
            ivals = np.zeros(GSIZE, dtype=np.int16)
            jvals = np.zeros(GSIZE, dtype=np.int16)
            lvals = np.zeros(GSIZE, dtype=np.float32)
            tvals = np.zeros(GSIZE, dtype=np.float32)
            ivals[:L] = (ep_c[ids, 0] >> 1).astype(np.int16)
            jvals[:L] = (ep_c[ids, 1] >> 1).astype(np.int16)
            lvals[:L] = len_c[ids]
            tvals[:L] = th_c[ids]
            ivals_pos = np.zeros(GSIZE, dtype=np.int16)
            jvals_pos = np.zeros(GSIZE, dtype=np.int16)
            lvals_pos = np.zeros(GSIZE, dtype=np.float32)
            tvals_pos = np.zeros(GSIZE, dtype=np.float32)
            ivals_pos[pos] = ivals
            jvals_pos[pos] = jvals
            lvals_pos[pos] = lvals
            tvals_pos[pos] = tvals
            col0 = 0
            for csz in chunks:
                idx_i[g, :, col0 // 16 : (col0 + csz) // 16] = _wrap_idx(ivals_pos[col0 : col0 + csz])
                idx_j[g, :, col0 // 16 : (col0 + csz) // 16] = _wrap_idx(jvals_pos[col0 : col0 + csz])
                col0 += csz
            lens_in[g] = lvals_pos.astype(BF16)
            trig_in[g, 0] = np.cos(tvals_pos).reshape(GSIZE // P, P).T
            trig_in[g, 1] = np.sin(tvals_pos).reshape(GSIZE // P, P).T
        m = dict(shared)
        m["idx_i"] = idx_i
        m["idx_j"] = idx_j
        m["lens"] = lens_in
        m["trig"] = trig_in
        in_maps.append(m)

    meta = {"E": E, "E_C": E_C, "eids": eids}
    return nc, in_maps, meta


def _assemble(results, meta):
    E, E_C, eids = meta["E"], meta["E_C"], meta["eids"]
    h0 = np.empty((E, SDIM), dtype=np.float32)
    v0 = np.empty((E, 2 * VDIM), dtype=np.float32)
    for c in range(N_CORES):
        h0c = np.asarray(results[c]["h0o"], dtype=np.float32)
        v0c = np.asarray(results[c]["v0o"], dtype=np.float32)
        base = c * E_C
        for g in range(4):
            ids = eids[c][g]
            L = len(ids)
            h0[base + ids] = h0c[g, :L]
            v0[base + ids] = v0c[g, :L]
    return h0, v0.reshape(E, VDIM, 2)


def kernel(**inputs):
    trace = inputs.pop("trace", False)
    nc, in_maps, meta = _prepare(**inputs)
    res = run_bass_kernel_spmd(nc, in_maps, core_ids=list(range(N_CORES)), trace=trace)
    kernel.last_result = res
    return _assemble(res.results, meta)
